# revision 36
# baseline (speedup 1.0000x reference)
"""Transformer block (pre-LN causal MHA + GELU MLP) on 8 trn2 NeuronCores.

Sharding: core r handles batch b=r//4, group position p=r%4, owning token
chunks {p, 7-p} of eight 256-token chunks (causally balanced zigzag).
Everything is sequence-parallel (zero duplicated flops) except attention:
K^T and V for the full batch are exchanged via AllGathers inside each
4-core batch group, split into two key-halves so attention on early keys
overlaps the second gather.

Attention computes transposed scores S^T[k, q] = K.Q^T so the softmax
row-sum falls out of a ones-augmented V matmul; no running max is needed
(|scores| <~ 8 for LN'd activations, exp is safe in fp32). Causal masks are
multiplicative 0/1 indicators built in-kernel from a tiny per-core qbase
input, so ONE SPMD program serves all 8 cores; head pairs are packed onto
the 128-partition axis (row-tiled K=64 matmuls) and the two phase-A query
chunks share 512-wide score/exp tiles.

Precision: QKV and the first MLP matmul run as split-fp8 DoubleRow GEMMs:
each bf16 operand is decomposed as hi + lo with both parts in fp8-e4m3, and
the K=256 DoubleRow mode evaluates hi*hi plus the two cross terms (the lo*lo
term is dropped), giving ~bf16 accuracy at 0.75 PE cycles per 128-deep
contraction column. Attention operands (K/V/Q/exp/W_o) and the second MLP
matmul are bf16 with fp32 PSUM accumulation; LN gamma/beta are folded into
the following weight matrix on the host, V's bias is folded into b_o.

Self-contained: hardcodes B=2, T=2048, C=1024, H=16, D=64, hidden=4096.
"""
import sys

if "/opt/trn_rl_repo" not in sys.path:
    sys.path.insert(0, "/opt/trn_rl_repo")

import numpy as np
import ml_dtypes

B, T, C, H = 2, 2048, 1024, 16
D = C // H            # 64
MH = 4 * C            # 4096 mlp hidden
EPS = 1e-5
P = 128
TOK = 512             # tokens per core
NCH = 256             # tokens per chunk
N_CORES = 8
SCALE = 1.0 / np.sqrt(D)
WS = 16.0             # fp8 weight pre-scale (host); descaled at PSUM read

_CACHE: dict = {}


def _split8(a):
    """two-term fp8-e4m3 split: a ~= hi + lo (elementwise)."""
    hi = np.asarray(a, np.float32).astype(ml_dtypes.float8_e4m3)
    lo = (np.asarray(a, np.float32) - hi.astype(np.float32)).astype(
        ml_dtypes.float8_e4m3)
    return hi, lo


def _pack_w8(w):
    """[C, N] float32 -> [128, C//128, 2, N] fp8 with comp order (lo, hi)."""
    cdim, n = w.shape
    kc = cdim // P
    ws = (w * WS).astype(np.float32)
    hi, lo = _split8(ws)
    out = np.empty((P, kc, 2, n), dtype=ml_dtypes.float8_e4m3)
    for c in range(kc):
        out[:, c, 0, :] = lo[c * P:(c + 1) * P, :]
        out[:, c, 1, :] = hi[c * P:(c + 1) * P, :]
    return out


def _build(mock_cc=False):
    import concourse.tile as tile
    from concourse import bacc, mybir
    from concourse.masks import make_identity
    from contextlib import ExitStack

    F32 = mybir.dt.float32
    BF16 = mybir.dt.bfloat16
    FP8 = mybir.dt.float8e4
    I32 = mybir.dt.int32
    AF = mybir.ActivationFunctionType
    ALU = mybir.AluOpType
    DR = mybir.MatmulPerfMode.DoubleRow

    nc = bacc.Bacc()

    # ---------------- I/O ----------------
    x_in = nc.declare_dram_parameter("x", [TOK, C], F32, isOutput=False)
    qbase_in = nc.declare_dram_parameter("qbase", [1, 2], F32, isOutput=False)
    w_attn8 = nc.declare_dram_parameter("w_attn8", [P, 8, 2, 3 * C], FP8, isOutput=False)
    b_attn = nc.declare_dram_parameter("b_attn", [3 * C], F32, isOutput=False)
    w_o = nc.declare_dram_parameter("w_o", [C, C], BF16, isOutput=False)
    b_o = nc.declare_dram_parameter("b_o", [C], F32, isOutput=False)
    w_fc8 = nc.declare_dram_parameter("w_fc8", [P, 8, 2, MH], FP8, isOutput=False)
    b_fc = nc.declare_dram_parameter("b_fc", [MH], F32, isOutput=False)
    w_fc2 = nc.declare_dram_parameter("w_fc2", [MH, C], BF16, isOutput=False)
    b_fc2 = nc.declare_dram_parameter("b_fc2", [C], F32, isOutput=False)
    out_ext = nc.declare_dram_parameter("out", [TOK, C], F32, isOutput=True)

    # internal DRAM for the collectives (A = keys 0:1024, B = keys 1024:2048)
    kt_in2 = [[nc.dram_tensor(f"kt_in_{s}_{h}", [512, NCH], BF16) for h in range(2)]
              for s in range(2)]
    v_in2 = [[nc.dram_tensor(f"v_in_{s}_{h}", [P, C], BF16) for h in range(2)]
             for s in range(2)]
    kt_g = [[nc.dram_tensor(f"kt_g_{s}_{h}", [4 * 512, NCH], BF16) for h in range(2)]
            for s in range(2)]
    v_g = [[nc.dram_tensor(f"v_g_{s}_{h}", [4 * P, C], BF16) for h in range(2)]
           for s in range(2)]
    stash_d = nc.dram_tensor("stash_d", [16, D + 1, NCH], F32)
    RG = [[0, 1, 2, 3], [4, 5, 6, 7]]

    with tile.TileContext(nc) as tc, ExitStack() as ctx:
        # ---------- pools: outer (whole kernel) ----------
        const = ctx.enter_context(tc.tile_pool(name="const", bufs=1))
        outer = ctx.enter_context(tc.tile_pool(name="outer", bufs=1))
        sm = ctx.enter_context(tc.tile_pool(name="sm", bufs=2))

        # ---------- constants ----------
        ident_bf = const.tile([P, P], BF16)
        make_identity(nc, ident_bf)
        eps_t = const.tile([P, 1], F32)
        nc.vector.memset(eps_t, EPS)
        ones128 = const.tile([P, P], F32)
        nc.vector.memset(ones128, 1.0)

        # per-feature bias tiles [128, 1] views
        bq_sb = const.tile([P, 8], F32)     # b_attn[0:1024]*SCALE -> [128, 8]
        nc.scalar.dma_start(out=bq_sb, in_=b_attn[0:C].rearrange("(f p) -> p f", p=P))
        bk_sb = const.tile([P, 8], F32)
        nc.scalar.dma_start(out=bk_sb, in_=b_attn[C:2 * C].rearrange("(f p) -> p f", p=P))
        bfc_sb = const.tile([P, 32], F32)
        nc.scalar.dma_start(out=bfc_sb, in_=b_fc[:].rearrange("(f p) -> p f", p=P))
        # broadcast bias tiles [128, C]
        bo_bc = const.tile([P, C], F32)
        nc.scalar.dma_start(out=bo_bc, in_=b_o[:].rearrange("(a c) -> a c", a=1).to_broadcast((P, C)))
        b2_bc = const.tile([P, C], F32)
        nc.scalar.dma_start(out=b2_bc, in_=b_fc2[:].rearrange("(a c) -> a c", a=1).to_broadcast((P, C)))

        # qbase for mask building (iota/compare ops emitted after LN1 so
        # they don't head-of-line-block the DVE queue at startup)
        qbase_sb = const.tile([1, 2], F32)
        nc.sync.dma_start(out=qbase_sb, in_=qbase_in[:, :])
        qk = []

        def build_qk():
            kidx_i = const.tile([P, 1], I32)
            nc.gpsimd.iota(kidx_i, pattern=[[0, 1]], base=0, channel_multiplier=1)
            kidx_f = const.tile([P, 1], F32)
            nc.vector.tensor_copy(out=kidx_f, in_=kidx_i)
            qio_i = const.tile([1, NCH], I32)
            nc.gpsimd.iota(qio_i, pattern=[[1, NCH]], base=0, channel_multiplier=0)
            qio_f = const.tile([1, NCH], F32)
            nc.vector.tensor_copy(out=qio_f, in_=qio_i)
            # qk[qc][k, q] = qglobal(qc, q) - k  (before subtracting 128*ktg)
            for qc in range(2):
                qg = const.tile([1, NCH], F32, name=f"qg{qc}")
                nc.vector.tensor_scalar_add(out=qg, in0=qio_f, scalar1=qbase_sb[0:1, qc:qc + 1])
                qgb = const.tile([P, NCH], F32, name=f"qgb{qc}")
                nc.gpsimd.partition_broadcast(qgb, qg)
                qkt = const.tile([P, NCH], F32, name=f"qk{qc}")
                nc.vector.tensor_scalar_sub(out=qkt, in0=qgb, scalar1=kidx_f)
                qk.append(qkt)

        # ---------- helpers ----------
        def layer_norm(src, dst_pool, tag):
            ln = dst_pool.tile([P, 4, C], BF16, name=tag, tag=tag)
            for t in range(4):
                stats = sm.tile([P, 2, 6], F32, name="lnstats", tag="lnstats")
                nc.vector.bn_stats(out=stats[:, 0, :], in_=src[:, t, 0:512])
                nc.vector.bn_stats(out=stats[:, 1, :], in_=src[:, t, 512:1024])
                mv = sm.tile([P, 2], F32, name="lnmv", tag="lnmv")
                nc.vector.bn_aggr(out=mv, in_=stats)
                rstd = sm.tile([P, 1], F32, name="lnrstd", tag="lnrstd")
                nc.scalar.activation(out=rstd, in_=mv[:, 1:2], func=AF.Sqrt, bias=eps_t, scale=1.0)
                nc.vector.reciprocal(out=rstd, in_=rstd)
                nc.vector.tensor_scalar(out=ln[:, t, :], in0=src[:, t, :],
                                        scalar1=mv[:, 0:1], scalar2=rstd,
                                        op0=ALU.subtract, op1=ALU.mult)
            return ln

        def transpose_split_to(lnt, dst_pool, dst_tag):
            """bf16 [P,4,C] -> fp8 hi/lo transposed [P, 8, 2, TOK]"""
            xt = dst_pool.tile([P, 8, 2, TOK], FP8, name=dst_tag, tag=dst_tag)
            with tc.tile_pool(name="tp_ps", bufs=2, space="PSUM") as tp_ps:
                for t in range(4):
                    for f in range(8):
                        pt = tp_ps.tile([P, P], BF16, name="tpt", tag="tpt",
                                        padded_shape=[P, 4 * P])
                        nc.tensor.transpose(pt[:, :], lnt[:, t, P * f:P * (f + 1)],
                                            ident_bf)
                        nc.scalar.activation(out=xt[:, f, 0, P * t:P * (t + 1)],
                                             in_=pt[:, :], func=AF.Identity, scale=1.0)
                        nc.vector.tensor_tensor(out=xt[:, f, 1, P * t:P * (t + 1)],
                                                in0=pt[:, :],
                                                in1=xt[:, f, 0, P * t:P * (t + 1)],
                                                op=ALU.subtract)
            return xt

        def s2_gemm(ps, w8, x8, col0, ncol):
            """psum += (w_hi+w_lo)^T (x_hi+x_lo) over K=1024, DoubleRow fp8.
            w8: [P, 8, 2(lo,hi), >=col0+ncol]; x8: [P, 8, 2(hi,lo), TOK]."""
            for kk in range(4):
                nc.tensor.matmul(ps, w8[:, 2 * kk:2 * kk + 2, 1, col0:col0 + ncol],
                                 x8[:, 2 * kk:2 * kk + 2, 0, :],
                                 start=(kk == 0), stop=False, perf_mode=DR)
            for k in range(8):
                nc.tensor.matmul(ps, w8[:, k, :, col0:col0 + ncol],
                                 x8[:, k, :, :],
                                 start=False, stop=(k == 7), perf_mode=DR)

        x2 = outer.tile([P, 4, C], F32)

        with tc.tile_pool(name="mid", bufs=1) as mid:
            x_sb = mid.tile([P, 4, C], F32)
            xq = [nc.sync, nc.scalar, nc.gpsimd, nc.sync]
            for t in range(4):
                xq[t].dma_start(out=x_sb[:, t, :], in_=x_in[P * t:P * (t + 1), :])
            qT = mid.tile([P, 8, TOK], BF16)
            yT = mid.tile([P, 8, TOK], BF16)

            # ================= qkv =================
            with tc.tile_pool(name="qkvp", bufs=1) as qp, \
                 tc.tile_pool(name="wqkv", bufs=2) as wp, \
                 tc.tile_pool(name="qkv_ps", bufs=3, space="PSUM") as qkv_ps:
                ln1 = layer_norm(x_sb, qp, "ln")
                x8 = transpose_split_to(ln1, qp, "x8")
                build_qk()

                def fire_kt(s, h):
                    if mock_cc:
                        nc.gpsimd.dma_start(out=kt_g[s][h][0:512, :], in_=kt_in2[s][h][:, :])
                    else:
                        nc.gpsimd.collective_compute("AllGather", ALU.bypass,
                                                     ins=[kt_in2[s][h][:, :]],
                                                     outs=[kt_g[s][h][:, :]],
                                                     replica_groups=RG)

                def fire_v(s, h):
                    if mock_cc:
                        nc.gpsimd.dma_start(out=v_g[s][h][0:P, :], in_=v_in2[s][h][:, :])
                    else:
                        nc.gpsimd.collective_compute("AllGather", ALU.bypass,
                                                     ins=[v_in2[s][h][:, :]],
                                                     outs=[v_g[s][h][:, :]],
                                                     replica_groups=RG)

                # K^T feature tiles -> kt_in halves
                for f in range(8):
                    if f % 4 == 0:
                        wk = wp.tile([P, 8, 2, 512], FP8, name="wk", tag="wk")
                        nc.sync.dma_start(out=wk, in_=w_attn8[:, :, :, C + 512 * (f // 4): C + 512 * (f // 4 + 1)])
                    fo = P * (f % 4)
                    ps = qkv_ps.tile([P, TOK], F32, name="kps", tag="qkvps")
                    s2_gemm(ps, wk, x8, fo, P)
                    kt_sb = sm.tile([P, TOK], BF16, name="kt_sb", tag="kt_sb", bufs=2)
                    nc.vector.tensor_scalar(out=kt_sb, in0=ps[:, :], scalar1=1.0 / WS,
                                            scalar2=bk_sb[:, f:f + 1],
                                            op0=ALU.mult, op1=ALU.add)
                    for s in range(2):
                        nc.sync.dma_start(out=kt_in2[s][f // 4][P * (f % 4):P * (f % 4 + 1), :],
                                          in_=kt_sb[:, NCH * s:NCH * (s + 1)])
                    if f == 3:
                        fire_kt(0, 0)
                    if f == 7:
                        fire_kt(0, 1)

                # V token tiles -> v_in halves (t-outer so the phase-A half
                # finishes after t=1 and the first AllGather can fire early)
                wvs = []
                for n in range(2):
                    wv = qp.tile([P, 8, 2, 512], FP8, name=f"wv{n}", tag=f"wv{n}")
                    nc.scalar.dma_start(out=wv, in_=w_attn8[:, :, :, 2 * C + 512 * n:2 * C + 512 * (n + 1)])
                    wvs.append(wv)
                for t in range(4):
                    for n in range(2):
                        ps = qkv_ps.tile([P, 512], F32, name="vps", tag="qkvps")
                        for kk in range(4):
                            nc.tensor.matmul(ps, x8[:, 2 * kk:2 * kk + 2, 0, P * t:P * (t + 1)],
                                             wvs[n][:, 2 * kk:2 * kk + 2, 1, :],
                                             start=(kk == 0), stop=False, perf_mode=DR)
                        for k in range(8):
                            nc.tensor.matmul(ps, x8[:, k, :, P * t:P * (t + 1)],
                                             wvs[n][:, k, :, :],
                                             start=False, stop=(k == 7), perf_mode=DR)
                        v_sb = sm.tile([P, 512], BF16, name="v_sb", tag="v_sb")
                        nc.vector.tensor_scalar(out=v_sb, in0=ps[:, :], scalar1=1.0 / WS,
                                                scalar2=None, op0=ALU.mult)
                        sh, row = divmod(t, 2)
                        nc.sync.dma_start(out=v_in2[sh][row][:, 512 * n:512 * (n + 1)],
                                          in_=v_sb)
                    fire_v(t // 2, t % 2)
                fire_kt(1, 0)
                fire_kt(1, 1)

                # Q^T feature tiles (stay local); fold in 1/sqrt(d) and 1/WS
                for f in range(8):
                    if f % 4 == 0:
                        wq = wp.tile([P, 8, 2, 512], FP8, name="wq", tag="wk")
                        nc.sync.dma_start(out=wq, in_=w_attn8[:, :, :, 512 * (f // 4): 512 * (f // 4 + 1)])
                    fo = P * (f % 4)
                    ps = qkv_ps.tile([P, TOK], F32, name="qps", tag="qkvps")
                    s2_gemm(ps, wq, x8, fo, P)
                    nc.vector.tensor_scalar(out=qT[:, f, :], in0=ps[:, :],
                                            scalar1=SCALE / WS, scalar2=bq_sb[:, f:f + 1],
                                            op0=ALU.mult, op1=ALU.add)

            # ============ attention (+ proj overlapped into phase B) ============
            with tc.tile_pool(name="attp", bufs=1) as ap, \
                 tc.tile_pool(name="projp", bufs=1) as pp, \
                 tc.tile_pool(name="pr_ps", bufs=2, space="PSUM") as pr_ps:
                wo_sb = pp.tile([P, 8, C], BF16)
                nc.sync.dma_start(out=wo_sb, in_=w_o[:, :].rearrange("(kc kp) n -> kp kc n", kp=P))
                for t in range(4):
                    nc.vector.tensor_tensor(out=x_sb[:, t, :], in0=x_sb[:, t, :], in1=bo_bc, op=ALU.add)

                def load_kv(s, eng):
                    ktb = ap.tile([P, 8, 4, NCH], BF16, name="ktb", tag="ktb", bufs=2)
                    vb = ap.tile([P, 8, 16, D + 1], BF16, name="vb", tag="vb", bufs=2)
                    for r in range(4):
                        blk = r if s == 0 else 3 - r     # rank block -> key slot
                        for h in range(2):
                            eng.dma_start(
                                out=ktb[:, 4 * h:4 * (h + 1), blk, :],
                                in_=kt_g[s][h][512 * r:512 * (r + 1), :]
                                        .rearrange("(j p) c -> p j c", p=P))
                        for sub in range(2):
                            eng.dma_start(
                                out=vb[:, 2 * blk + sub, :, 0:D],
                                in_=v_g[s][sub][P * r:P * (r + 1), :]
                                        .rearrange("p (h d) -> p h d", h=H))
                    nc.vector.tensor_copy(out=vb[:, :, :, D:D + 1],
                                          in_=ones128.rearrange("p (a b) -> p a b", a=8)[:, :, 0:16])
                    return ktb, vb

                def build_ind(s, qc):
                    ind = sm.tile([P, 8, 2, NCH], BF16, name="ind", tag="ind", bufs=1)
                    for kt in range(8):
                        ktg = 8 * s + kt
                        for i in range(2):
                            nc.vector.tensor_scalar(out=ind[:, kt, i, :], in0=qk[qc],
                                                    scalar1=float(P * ktg), scalar2=None,
                                                    op0=ALU.is_ge)
                    return ind

                def div_write(ya_h, h, j, qc, ysrc, rsrc):
                    recip = sm.tile([1, NCH], F32, name=f"rc{h}", tag=f"rc{h}")
                    nc.vector.reciprocal(out=recip, in_=rsrc)
                    rb = sm.tile([D, NCH], F32, name=f"rb{h}", tag=f"rb{h}")
                    nc.gpsimd.partition_broadcast(rb, recip)
                    nc.vector.tensor_tensor(out=yT[64 * h:64 * (h + 1), j, NCH * qc:NCH * (qc + 1)],
                                            in0=ysrc, in1=rb, op=ALU.mult)

                def proj(trange):
                    for t in trange:
                        for n in range(2):
                            ps = pr_ps.tile([P, 512], F32, name="prps", tag="prps")
                            for k in range(8):
                                nc.tensor.matmul(ps[:, :], yT[:, k, P * t:P * (t + 1)],
                                                 wo_sb[:, k, 512 * n:512 * (n + 1)],
                                                 start=(k == 0), stop=(k == 7))
                            nc.vector.tensor_tensor(out=x2[:, t, 512 * n:512 * (n + 1)], in0=ps[:, :],
                                                    in1=x_sb[:, t, 512 * n:512 * (n + 1)], op=ALU.add)

                # ---- load/build both phases up front: phase-B tiles land while
                # phase-A computes (hides the second AllGather + load bubble) ----
                ktb, vb = load_kv(0, nc.scalar)
                ind = build_ind(0, 0)     # only chunk 0 can be non-causal here
                ktbB, vbB = load_kv(1, nc.sync)
                with tc.tile_pool(name="at_ps0", bufs=1, space="PSUM") as at_ps:
                    for j in range(8):
                        ya = [at_ps.tile([D + 1, TOK], F32, name=f"ya{h}", tag=f"ya{h}", bufs=1)
                              for h in range(2)]
                        for kt in range(8):
                            st = at_ps.tile([P, 2, TOK], F32, name="st", tag="st", bufs=2)
                            for h in range(2):
                                nc.tensor.matmul(
                                    st[:, h, :],
                                    ktb[64 * h:64 * (h + 1), j, kt // 2, (kt % 2) * P:(kt % 2) * P + P],
                                    qT[64 * h:64 * (h + 1), j, :],
                                    start=True, stop=True, tile_position=(64 * h, 0))
                            et = sm.tile([P, 2, TOK], BF16, name="et", tag="et", bufs=3)
                            nc.scalar.activation(out=et, in_=st[:, :, :], func=AF.Exp, scale=1.0)
                            nc.vector.tensor_tensor(out=et[:, :, 0:NCH], in0=et[:, :, 0:NCH],
                                                    in1=ind[:, kt, :, :], op=ALU.mult)
                            for h in range(2):
                                nc.tensor.matmul(ya[h][:, :], vb[:, kt, 2 * j + h, :], et[:, h, :],
                                                 start=(kt == 0), stop=(kt == 7))
                        for h in range(2):
                            hh = 2 * j + h
                            stc = sm.tile([D + 1, NCH], F32, name=f"stc{h}", tag=f"stc{h}")
                            nc.vector.tensor_copy(out=stc, in_=ya[h][:, NCH:TOK])
                            nc.sync.dma_start(out=stash_d[hh, :, :], in_=stc)
                            div_write(ya, h, j, 0, ya[h][0:D, 0:NCH], ya[h][D:D + 1, 0:NCH])

                proj([0, 1])

                # ---- phase B: keys 1024:2047, chunk 1 only ----
                ktb, vb = ktbB, vbB
                ind = build_ind(1, 1)
                with tc.tile_pool(name="at_ps1", bufs=1, space="PSUM") as at_ps:
                    for j in range(8):
                        ya = [at_ps.tile([D + 1, NCH], F32, name=f"ya{h}", tag=f"ya{h}", bufs=1)
                              for h in range(2)]
                        for kt in range(8):
                            st = at_ps.tile([P, 2, TOK], F32, name="st", tag="st", bufs=2)
                            for h in range(2):
                                nc.tensor.matmul(
                                    st[:, h, 0:NCH],
                                    ktb[64 * h:64 * (h + 1), j, kt // 2, (kt % 2) * P:(kt % 2) * P + P],
                                    qT[64 * h:64 * (h + 1), j, NCH:TOK],
                                    start=True, stop=True, tile_position=(64 * h, 0))
                            et = sm.tile([P, 2, NCH], BF16, name="etb", tag="etb", bufs=3)
                            nc.scalar.activation(out=et, in_=st[:, :, 0:NCH], func=AF.Exp, scale=1.0)
                            nc.vector.tensor_tensor(out=et[:, :, :], in0=et[:, :, :],
                                                    in1=ind[:, kt, :, :], op=ALU.mult)
                            for h in range(2):
                                nc.tensor.matmul(ya[h][:, :], vb[:, kt, 2 * j + h, :], et[:, h, :],
                                                 start=(kt == 0), stop=(kt == 7))
                        for h in range(2):
                            hh = 2 * j + h
                            stl = sm.tile([D + 1, NCH], F32, name=f"stl{h}", tag=f"stl{h}", bufs=1)
                            nc.sync.dma_start(out=stl, in_=stash_d[hh, :, :])
                            ysum = sm.tile([D + 1, NCH], F32, name=f"ys{h}", tag=f"ys{h}")
                            nc.vector.tensor_tensor(out=ysum, in0=ya[h][:, :], in1=stl, op=ALU.add)
                            div_write(ya, h, j, 1, ysum[0:D, :], ysum[D:D + 1, :])

                proj([2, 3])

        # ================= LN2 + MLP =================
        with tc.tile_pool(name="mlpp", bufs=1) as mp, \
             tc.tile_pool(name="wmlp", bufs=3) as wmp:
            ln2 = layer_norm(x2, mp, "ln2")
            x28 = transpose_split_to(ln2, mp, "x28")
            for t in range(4):
                nc.vector.tensor_tensor(out=x2[:, t, :], in0=x2[:, t, :], in1=b2_bc, op=ALU.add)

            h_sb = mp.tile([P, 32, 512], BF16)
            for half in range(2):
                with tc.tile_pool(name=f"mlp_ps{half}", bufs=1, space="PSUM") as mlp_ps:
                    ops = [mlp_ps.tile([P, 512], F32, name=f"ops{t}", tag=f"ops{t}", bufs=1)
                           for t in range(4)]
                    for m in range(32):
                        if half == 0:
                            if m % 4 == 0:
                                wfc = wmp.tile([P, 8, 2, 512], FP8, name="wfc", tag="wfc")
                                nc.sync.dma_start(out=wfc,
                                                    in_=w_fc8[:, :, :, 512 * (m // 4):512 * (m // 4 + 1)])
                            mo = P * (m % 4)
                            fps = mlp_ps.tile([P, 512], F32, name="fps", tag="fps", bufs=4)
                            s2_gemm(fps, wfc, x28, mo, P)
                            nc.scalar.activation(out=h_sb[:, m, :], in_=fps[:, :], func=AF.Gelu,
                                                 bias=bfc_sb[:, m:m + 1], scale=1.0 / WS)
                        if m % 4 == 0:
                            w2 = wmp.tile([P, 4, 512], BF16, name="w2", tag="w2", bufs=3)
                            nc.gpsimd.dma_start(out=w2, in_=w_fc2[P * m:P * (m + 4),
                                                               512 * half:512 * (half + 1)]
                                                .rearrange("(mc mp) n -> mp mc n", mp=P))
                        for t in range(4):
                            nc.tensor.matmul(ops[t][:, :], h_sb[:, m, P * t:P * (t + 1)],
                                             w2[:, m % 4, :], start=(m == 0), stop=(m == 31))
                    for t in range(4):
                        nc.vector.tensor_tensor(out=x2[:, t, 512 * half:512 * (half + 1)],
                                                in0=ops[t][:, :],
                                                in1=x2[:, t, 512 * half:512 * (half + 1)], op=ALU.add)
                        if half == 1:
                            nc.sync.dma_start(out=out_ext[P * t:P * (t + 1), :], in_=x2[:, t, :])

    nc.finalize()
    return nc


def _get_nc():
    if "nc" not in _CACHE:
        _CACHE["nc"] = _build()
    return _CACHE["nc"]


def _prep(**inputs):
    f = lambda a: np.asarray(a, dtype=np.float32)
    x = f(inputs["x"])
    ln1_g, ln1_b = f(inputs["ln1_g"]), f(inputs["ln1_b"])
    ln2_g, ln2_b = f(inputs["ln2_g"]), f(inputs["ln2_b"])
    W_attn, b_attn = f(inputs["W_attn"]), f(inputs["b_attn"])
    W_o, b_o = f(inputs["W_o"]), f(inputs["b_o"])
    W_fc, b_fc = f(inputs["W_fc"]), f(inputs["b_fc"])
    W_fc2, b_fc2 = f(inputs["W_fc2"]), f(inputs["b_fc2"])

    # fold LN affine params into the next matmul
    W_attn_e = ln1_g[:, None] * W_attn
    b_attn_e = b_attn + ln1_b @ W_attn
    W_fc_e = ln2_g[:, None] * W_fc
    b_fc_e = b_fc + ln2_b @ W_fc

    # fold V bias into b_o (sum of attention weights is 1); pre-scale Q bias
    b_v = b_attn_e[2 * C:3 * C]
    b_o_e = b_o + b_v @ W_o
    b_attn_pass = b_attn_e.copy()
    b_attn_pass[0:C] *= SCALE

    w_attn8 = _pack_w8(W_attn_e)
    w_fc8 = _pack_w8(W_fc_e)

    in_maps = []
    for r in range(N_CORES):
        b, p = divmod(r, 4)
        c0, c1 = p, 7 - p
        xs = np.concatenate([x[b, NCH * c0:NCH * (c0 + 1)],
                             x[b, NCH * c1:NCH * (c1 + 1)]], axis=0)
        in_maps.append({
            "x": np.ascontiguousarray(xs),
            "qbase": np.array([[NCH * c0, NCH * c1]], dtype=np.float32),
            "w_attn8": w_attn8, "b_attn": b_attn_pass,
            "w_o": W_o.astype(ml_dtypes.bfloat16), "b_o": b_o_e,
            "w_fc8": w_fc8, "b_fc": b_fc_e,
            "w_fc2": W_fc2.astype(ml_dtypes.bfloat16), "b_fc2": b_fc2,
        })

    def assemble(results):
        out = np.empty((B, T, C), dtype=np.float32)
        for r in range(N_CORES):
            b, p = divmod(r, 4)
            c0, c1 = p, 7 - p
            o = results[r]["out"]
            out[b, NCH * c0:NCH * (c0 + 1)] = o[0:NCH]
            out[b, NCH * c1:NCH * (c1 + 1)] = o[NCH:TOK]
        return out

    return in_maps, assemble


def kernel(**inputs):
    from concourse.bass_utils import run_bass_kernel_spmd

    in_maps, assemble = _prep(**inputs)
    res = run_bass_kernel_spmd(_get_nc(), in_maps, list(range(N_CORES)))
    return assemble(res.results)


# revision 39
# speedup vs baseline: 1.0220x; 1.0220x over previous
"""Transformer block (pre-LN causal MHA + GELU MLP) on 8 trn2 NeuronCores.

Sharding: core r handles batch b=r//4, group position p=r%4, owning token
chunks {p, 7-p} of eight 256-token chunks (causally balanced zigzag).
Everything is sequence-parallel (zero duplicated flops) except attention:
K^T and V for the full batch are exchanged via AllGathers inside each
4-core batch group, split into two key-halves so attention on early keys
overlaps the second gather.

Attention computes transposed scores S^T[k, q] = K.Q^T so the softmax
row-sum falls out of a ones-augmented V matmul; no running max is needed
(|scores| <~ 8 for LN'd activations, exp is safe in fp32). Causal masks are
multiplicative 0/1 indicators built in-kernel from a tiny per-core qbase
input, so ONE SPMD program serves all 8 cores; head pairs are packed onto
the 128-partition axis (row-tiled K=64 matmuls) and the two phase-A query
chunks share 512-wide score/exp tiles.

Precision: QKV and the first MLP matmul run as split-fp8 DoubleRow GEMMs:
each bf16 operand is decomposed as hi + lo with both parts in fp8-e4m3, and
the K=256 DoubleRow mode evaluates hi*hi plus the two cross terms (the lo*lo
term is dropped), giving ~bf16 accuracy at 0.75 PE cycles per 128-deep
contraction column. Attention operands (K/V/Q/exp/W_o) and the second MLP
matmul are bf16 with fp32 PSUM accumulation; LN gamma/beta are folded into
the following weight matrix on the host, V's bias is folded into b_o.

Self-contained: hardcodes B=2, T=2048, C=1024, H=16, D=64, hidden=4096.
"""
import sys

if "/opt/trn_rl_repo" not in sys.path:
    sys.path.insert(0, "/opt/trn_rl_repo")

import numpy as np
import ml_dtypes

B, T, C, H = 2, 2048, 1024, 16
D = C // H            # 64
MH = 4 * C            # 4096 mlp hidden
EPS = 1e-5
P = 128
TOK = 512             # tokens per core
NCH = 256             # tokens per chunk
N_CORES = 8
SCALE = 1.0 / np.sqrt(D)
WS = 16.0             # fp8 weight pre-scale (host); descaled at PSUM read

_CACHE: dict = {}


def _split8(a):
    """two-term fp8-e4m3 split: a ~= hi + lo (elementwise)."""
    hi = np.asarray(a, np.float32).astype(ml_dtypes.float8_e4m3)
    lo = (np.asarray(a, np.float32) - hi.astype(np.float32)).astype(
        ml_dtypes.float8_e4m3)
    return hi, lo


def _pack_w8(w):
    """[C, N] float32 -> [128, C//128, 2, N] fp8 with comp order (lo, hi)."""
    cdim, n = w.shape
    kc = cdim // P
    ws = (w * WS).astype(np.float32)
    hi, lo = _split8(ws)
    out = np.empty((P, kc, 2, n), dtype=ml_dtypes.float8_e4m3)
    for c in range(kc):
        out[:, c, 0, :] = lo[c * P:(c + 1) * P, :]
        out[:, c, 1, :] = hi[c * P:(c + 1) * P, :]
    return out


def _build(mock_cc=False):
    import concourse.tile as tile
    from concourse import bacc, mybir
    from concourse.masks import make_identity
    from contextlib import ExitStack

    F32 = mybir.dt.float32
    BF16 = mybir.dt.bfloat16
    FP8 = mybir.dt.float8e4
    I32 = mybir.dt.int32
    AF = mybir.ActivationFunctionType
    ALU = mybir.AluOpType
    DR = mybir.MatmulPerfMode.DoubleRow

    nc = bacc.Bacc()

    # ---------------- I/O ----------------
    x_in = nc.declare_dram_parameter("x", [TOK, C], F32, isOutput=False)
    qbase_in = nc.declare_dram_parameter("qbase", [1, 2], F32, isOutput=False)
    w_attn8 = nc.declare_dram_parameter("w_attn8", [P, 8, 2, 3 * C], FP8, isOutput=False)
    b_attn = nc.declare_dram_parameter("b_attn", [3 * C], F32, isOutput=False)
    w_o = nc.declare_dram_parameter("w_o", [C, C], BF16, isOutput=False)
    b_o = nc.declare_dram_parameter("b_o", [C], F32, isOutput=False)
    w_fc8 = nc.declare_dram_parameter("w_fc8", [P, 8, 2, MH], FP8, isOutput=False)
    b_fc = nc.declare_dram_parameter("b_fc", [MH], F32, isOutput=False)
    w_fc2 = nc.declare_dram_parameter("w_fc2", [MH, C], BF16, isOutput=False)
    b_fc2 = nc.declare_dram_parameter("b_fc2", [C], F32, isOutput=False)
    out_ext = nc.declare_dram_parameter("out", [TOK, C], F32, isOutput=True)

    # internal DRAM for the collectives (A = keys 0:1024, B = keys 1024:2048)
    kt_in2 = [[nc.dram_tensor(f"kt_in_{s}_{h}", [512, NCH], BF16) for h in range(2)]
              for s in range(2)]
    v_in2 = [[nc.dram_tensor(f"v_in_{s}_{h}", [P, C], BF16) for h in range(2)]
             for s in range(2)]
    kt_g = [[nc.dram_tensor(f"kt_g_{s}_{h}", [4 * 512, NCH], BF16) for h in range(2)]
            for s in range(2)]
    v_g = [[nc.dram_tensor(f"v_g_{s}_{h}", [4 * P, C], BF16) for h in range(2)]
           for s in range(2)]
    stash_d = nc.dram_tensor("stash_d", [16, D + 1, NCH], F32)
    RG = [[0, 1, 2, 3], [4, 5, 6, 7]]

    with tile.TileContext(nc) as tc, ExitStack() as ctx:
        # ---------- pools: outer (whole kernel) ----------
        const = ctx.enter_context(tc.tile_pool(name="const", bufs=1))
        outer = ctx.enter_context(tc.tile_pool(name="outer", bufs=1))
        sm = ctx.enter_context(tc.tile_pool(name="sm", bufs=2))

        # ---------- constants ----------
        ident_bf = const.tile([P, P], BF16)
        make_identity(nc, ident_bf)
        eps_t = const.tile([P, 1], F32)
        nc.vector.memset(eps_t, EPS)
        ones128 = const.tile([P, P], F32)
        nc.vector.memset(ones128, 1.0)

        # per-feature bias tiles [128, 1] views
        bq_sb = const.tile([P, 8], F32)     # b_attn[0:1024]*SCALE -> [128, 8]
        nc.scalar.dma_start(out=bq_sb, in_=b_attn[0:C].rearrange("(f p) -> p f", p=P))
        bk_sb = const.tile([P, 8], F32)
        nc.scalar.dma_start(out=bk_sb, in_=b_attn[C:2 * C].rearrange("(f p) -> p f", p=P))
        bfc_sb = const.tile([P, 32], F32)
        nc.scalar.dma_start(out=bfc_sb, in_=b_fc[:].rearrange("(f p) -> p f", p=P))
        # broadcast bias tiles [128, C]
        bo_bc = const.tile([P, C], F32)
        nc.scalar.dma_start(out=bo_bc, in_=b_o[:].rearrange("(a c) -> a c", a=1).to_broadcast((P, C)))
        b2_bc = const.tile([P, C], F32)
        nc.scalar.dma_start(out=b2_bc, in_=b_fc2[:].rearrange("(a c) -> a c", a=1).to_broadcast((P, C)))

        # qbase for mask building (iota/compare ops emitted after LN1 so
        # they don't head-of-line-block the DVE queue at startup)
        qbase_sb = const.tile([1, 2], F32)
        nc.sync.dma_start(out=qbase_sb, in_=qbase_in[:, :])
        qk = []

        def build_qk():
            kidx_i = const.tile([P, 1], I32)
            nc.gpsimd.iota(kidx_i, pattern=[[0, 1]], base=0, channel_multiplier=1)
            kidx_f = const.tile([P, 1], F32)
            nc.vector.tensor_copy(out=kidx_f, in_=kidx_i)
            qio_i = const.tile([1, NCH], I32)
            nc.gpsimd.iota(qio_i, pattern=[[1, NCH]], base=0, channel_multiplier=0)
            qio_f = const.tile([1, NCH], F32)
            nc.vector.tensor_copy(out=qio_f, in_=qio_i)
            # qk[qc][k, q] = qglobal(qc, q) - k  (before subtracting 128*ktg)
            for qc in range(2):
                qg = const.tile([1, NCH], F32, name=f"qg{qc}")
                nc.vector.tensor_scalar_add(out=qg, in0=qio_f, scalar1=qbase_sb[0:1, qc:qc + 1])
                qgb = const.tile([P, NCH], F32, name=f"qgb{qc}")
                nc.gpsimd.partition_broadcast(qgb, qg)
                qkt = const.tile([P, NCH], F32, name=f"qk{qc}")
                nc.vector.tensor_scalar_sub(out=qkt, in0=qgb, scalar1=kidx_f)
                qk.append(qkt)

        # ---------- helpers ----------
        def layer_norm(src, dst_pool, tag):
            ln = dst_pool.tile([P, 4, C], BF16, name=tag, tag=tag)
            for t in range(4):
                stats = sm.tile([P, 2, 6], F32, name="lnstats", tag="lnstats")
                nc.vector.bn_stats(out=stats[:, 0, :], in_=src[:, t, 0:512])
                nc.vector.bn_stats(out=stats[:, 1, :], in_=src[:, t, 512:1024])
                mv = sm.tile([P, 2], F32, name="lnmv", tag="lnmv")
                nc.vector.bn_aggr(out=mv, in_=stats)
                rstd = sm.tile([P, 1], F32, name="lnrstd", tag="lnrstd")
                nc.scalar.activation(out=rstd, in_=mv[:, 1:2], func=AF.Sqrt, bias=eps_t, scale=1.0)
                nc.vector.reciprocal(out=rstd, in_=rstd)
                nc.vector.tensor_scalar(out=ln[:, t, :], in0=src[:, t, :],
                                        scalar1=mv[:, 0:1], scalar2=rstd,
                                        op0=ALU.subtract, op1=ALU.mult)
            return ln

        def transpose_split_to(lnt, dst_pool, dst_tag):
            """bf16 [P,4,C] -> fp8 hi/lo transposed [P, 8, 2, TOK]"""
            xt = dst_pool.tile([P, 8, 2, TOK], FP8, name=dst_tag, tag=dst_tag)
            with tc.tile_pool(name="tp_ps", bufs=2, space="PSUM") as tp_ps:
                for t in range(4):
                    for f in range(8):
                        pt = tp_ps.tile([P, P], BF16, name="tpt", tag="tpt",
                                        padded_shape=[P, 4 * P])
                        nc.tensor.transpose(pt[:, :], lnt[:, t, P * f:P * (f + 1)],
                                            ident_bf)
                        nc.scalar.activation(out=xt[:, f, 0, P * t:P * (t + 1)],
                                             in_=pt[:, :], func=AF.Identity, scale=1.0)
                        nc.vector.tensor_tensor(out=xt[:, f, 1, P * t:P * (t + 1)],
                                                in0=pt[:, :],
                                                in1=xt[:, f, 0, P * t:P * (t + 1)],
                                                op=ALU.subtract)
            return xt

        def s2_gemm(ps, w8, x8, col0, ncol):
            """psum += (w_hi+w_lo)^T (x_hi+x_lo) over K=1024, DoubleRow fp8.
            w8: [P, 8, 2(lo,hi), >=col0+ncol]; x8: [P, 8, 2(hi,lo), TOK]."""
            for kk in range(4):
                nc.tensor.matmul(ps, w8[:, 2 * kk:2 * kk + 2, 1, col0:col0 + ncol],
                                 x8[:, 2 * kk:2 * kk + 2, 0, :],
                                 start=(kk == 0), stop=False, perf_mode=DR)
            for k in range(8):
                nc.tensor.matmul(ps, w8[:, k, :, col0:col0 + ncol],
                                 x8[:, k, :, :],
                                 start=False, stop=(k == 7), perf_mode=DR)

        x2 = outer.tile([P, 4, C], F32)

        with tc.tile_pool(name="mid", bufs=1) as mid:
            x_sb = mid.tile([P, 4, C], F32)
            xq = [nc.sync, nc.scalar, nc.gpsimd, nc.sync]
            for t in range(4):
                xq[t].dma_start(out=x_sb[:, t, :], in_=x_in[P * t:P * (t + 1), :])
            qT = mid.tile([P, 8, TOK], BF16)
            yT = mid.tile([P, 8, TOK], BF16)

            # ================= qkv =================
            with tc.tile_pool(name="qkvp", bufs=1) as qp, \
                 tc.tile_pool(name="wqkv", bufs=2) as wp, \
                 tc.tile_pool(name="qkv_ps", bufs=3, space="PSUM") as qkv_ps:
                ln1 = layer_norm(x_sb, qp, "ln")
                x8 = transpose_split_to(ln1, qp, "x8")
                build_qk()

                def fire_kt(s, h):
                    if mock_cc:
                        nc.gpsimd.dma_start(out=kt_g[s][h][0:512, :], in_=kt_in2[s][h][:, :])
                    else:
                        nc.gpsimd.collective_compute("AllGather", ALU.bypass,
                                                     ins=[kt_in2[s][h][:, :]],
                                                     outs=[kt_g[s][h][:, :]],
                                                     replica_groups=RG)

                def fire_v(s, h):
                    if mock_cc:
                        nc.gpsimd.dma_start(out=v_g[s][h][0:P, :], in_=v_in2[s][h][:, :])
                    else:
                        nc.gpsimd.collective_compute("AllGather", ALU.bypass,
                                                     ins=[v_in2[s][h][:, :]],
                                                     outs=[v_g[s][h][:, :]],
                                                     replica_groups=RG)

                # K^T feature tiles -> kt_in halves
                for f in range(8):
                    if f % 4 == 0:
                        wk = wp.tile([P, 8, 2, 512], FP8, name="wk", tag="wk")
                        nc.sync.dma_start(out=wk, in_=w_attn8[:, :, :, C + 512 * (f // 4): C + 512 * (f // 4 + 1)])
                    fo = P * (f % 4)
                    ps = qkv_ps.tile([P, TOK], F32, name="kps", tag="qkvps")
                    s2_gemm(ps, wk, x8, fo, P)
                    kt_sb = sm.tile([P, TOK], BF16, name="kt_sb", tag="kt_sb", bufs=2)
                    nc.vector.tensor_scalar(out=kt_sb, in0=ps[:, :], scalar1=1.0 / WS,
                                            scalar2=bk_sb[:, f:f + 1],
                                            op0=ALU.mult, op1=ALU.add)
                    for s in range(2):
                        nc.sync.dma_start(out=kt_in2[s][f // 4][P * (f % 4):P * (f % 4 + 1), :],
                                          in_=kt_sb[:, NCH * s:NCH * (s + 1)])
                    if f == 3:
                        fire_kt(0, 0)
                    if f == 7:
                        fire_kt(0, 1)

                # V token tiles -> v_in halves (t-outer so the phase-A half
                # finishes after t=1 and the first AllGather can fire early)
                wvs = []
                for n in range(2):
                    wv = qp.tile([P, 8, 2, 512], FP8, name=f"wv{n}", tag=f"wv{n}")
                    nc.scalar.dma_start(out=wv, in_=w_attn8[:, :, :, 2 * C + 512 * n:2 * C + 512 * (n + 1)])
                    wvs.append(wv)
                for t in range(4):
                    for n in range(2):
                        ps = qkv_ps.tile([P, 512], F32, name="vps", tag="qkvps")
                        for kk in range(4):
                            nc.tensor.matmul(ps, x8[:, 2 * kk:2 * kk + 2, 0, P * t:P * (t + 1)],
                                             wvs[n][:, 2 * kk:2 * kk + 2, 1, :],
                                             start=(kk == 0), stop=False, perf_mode=DR)
                        for k in range(8):
                            nc.tensor.matmul(ps, x8[:, k, :, P * t:P * (t + 1)],
                                             wvs[n][:, k, :, :],
                                             start=False, stop=(k == 7), perf_mode=DR)
                        v_sb = sm.tile([P, 512], BF16, name="v_sb", tag="v_sb")
                        nc.vector.tensor_scalar(out=v_sb, in0=ps[:, :], scalar1=1.0 / WS,
                                                scalar2=None, op0=ALU.mult)
                        sh, row = divmod(t, 2)
                        nc.sync.dma_start(out=v_in2[sh][row][:, 512 * n:512 * (n + 1)],
                                          in_=v_sb)
                    fire_v(t // 2, t % 2)
                fire_kt(1, 0)
                fire_kt(1, 1)

                # Q^T feature tiles (stay local); fold in 1/sqrt(d) and 1/WS
                for f in range(8):
                    if f % 4 == 0:
                        wq = wp.tile([P, 8, 2, 512], FP8, name="wq", tag="wk")
                        nc.sync.dma_start(out=wq, in_=w_attn8[:, :, :, 512 * (f // 4): 512 * (f // 4 + 1)])
                    fo = P * (f % 4)
                    ps = qkv_ps.tile([P, TOK], F32, name="qps", tag="qkvps")
                    s2_gemm(ps, wq, x8, fo, P)
                    nc.vector.tensor_scalar(out=qT[:, f, :], in0=ps[:, :],
                                            scalar1=SCALE / WS, scalar2=bq_sb[:, f:f + 1],
                                            op0=ALU.mult, op1=ALU.add)

            # ============ attention (+ proj overlapped into phase B) ============
            with tc.tile_pool(name="attp", bufs=1) as ap, \
                 tc.tile_pool(name="projp", bufs=1) as pp, \
                 tc.tile_pool(name="pr_ps", bufs=2, space="PSUM") as pr_ps:
                wo_sb = pp.tile([P, 8, C], BF16)
                nc.sync.dma_start(out=wo_sb, in_=w_o[:, :].rearrange("(kc kp) n -> kp kc n", kp=P))
                for t in range(4):
                    nc.vector.tensor_tensor(out=x_sb[:, t, :], in0=x_sb[:, t, :], in1=bo_bc, op=ALU.add)

                def load_kv(s, eng):
                    ktb = ap.tile([P, 8, 4, NCH], BF16, name="ktb", tag="ktb", bufs=2)
                    vb = ap.tile([P, 8, 16, D + 1], BF16, name="vb", tag="vb", bufs=2)
                    for r in range(4):
                        blk = r if s == 0 else 3 - r     # rank block -> key slot
                        for h in range(2):
                            eng.dma_start(
                                out=ktb[:, 4 * h:4 * (h + 1), blk, :],
                                in_=kt_g[s][h][512 * r:512 * (r + 1), :]
                                        .rearrange("(j p) c -> p j c", p=P))
                        for sub in range(2):
                            eng.dma_start(
                                out=vb[:, 2 * blk + sub, :, 0:D],
                                in_=v_g[s][sub][P * r:P * (r + 1), :]
                                        .rearrange("p (h d) -> p h d", h=H))
                    nc.vector.tensor_copy(out=vb[:, :, :, D:D + 1],
                                          in_=ones128.rearrange("p (a b) -> p a b", a=8)[:, :, 0:16])
                    return ktb, vb

                def build_ind(s, qc):
                    ind = sm.tile([P, 8, 2, NCH], BF16, name="ind", tag="ind", bufs=1)
                    for kt in range(8):
                        ktg = 8 * s + kt
                        for i in range(2):
                            nc.vector.tensor_scalar(out=ind[:, kt, i, :], in0=qk[qc],
                                                    scalar1=float(P * ktg), scalar2=None,
                                                    op0=ALU.is_ge)
                    return ind

                def div_write(ya_h, h, j, qc, ysrc, rsrc):
                    recip = sm.tile([1, NCH], F32, name=f"rc{h}", tag=f"rc{h}")
                    nc.vector.reciprocal(out=recip, in_=rsrc)
                    rb = sm.tile([D, NCH], F32, name=f"rb{h}", tag=f"rb{h}")
                    nc.gpsimd.partition_broadcast(rb, recip)
                    nc.vector.tensor_tensor(out=yT[64 * h:64 * (h + 1), j, NCH * qc:NCH * (qc + 1)],
                                            in0=ysrc, in1=rb, op=ALU.mult)

                def proj(trange):
                    for t in trange:
                        for n in range(2):
                            ps = pr_ps.tile([P, 512], F32, name="prps", tag="prps")
                            for k in range(8):
                                nc.tensor.matmul(ps[:, :], yT[:, k, P * t:P * (t + 1)],
                                                 wo_sb[:, k, 512 * n:512 * (n + 1)],
                                                 start=(k == 0), stop=(k == 7))
                            nc.vector.tensor_tensor(out=x2[:, t, 512 * n:512 * (n + 1)], in0=ps[:, :],
                                                    in1=x_sb[:, t, 512 * n:512 * (n + 1)], op=ALU.add)

                # ---- load/build both phases up front: phase-B tiles land while
                # phase-A computes (hides the second AllGather + load bubble) ----
                ktb, vb = load_kv(0, nc.scalar)
                ind = build_ind(0, 0)     # only chunk 0 can be non-causal here
                ktbB, vbB = load_kv(1, nc.sync)
                with tc.tile_pool(name="at_ps0", bufs=1, space="PSUM") as at_ps:
                    for j in range(8):
                        ya = [at_ps.tile([D + 1, TOK], F32, name=f"ya{h}", tag=f"ya{h}", bufs=1)
                              for h in range(2)]
                        for kt in range(8):
                            st = at_ps.tile([P, 2, TOK], F32, name="st", tag="st", bufs=2)
                            for h in range(2):
                                nc.tensor.matmul(
                                    st[:, h, :],
                                    ktb[64 * h:64 * (h + 1), j, kt // 2, (kt % 2) * P:(kt % 2) * P + P],
                                    qT[64 * h:64 * (h + 1), j, :],
                                    start=True, stop=True, tile_position=(64 * h, 0))
                            et = sm.tile([P, 2, TOK], BF16, name="et", tag="et", bufs=3)
                            nc.scalar.activation(out=et, in_=st[:, :, :], func=AF.Exp, scale=1.0)
                            nc.vector.tensor_tensor(out=et[:, :, 0:NCH], in0=et[:, :, 0:NCH],
                                                    in1=ind[:, kt, :, :], op=ALU.mult)
                            for h in range(2):
                                nc.tensor.matmul(ya[h][:, :], vb[:, kt, 2 * j + h, :], et[:, h, :],
                                                 start=(kt == 0), stop=(kt == 7))
                        for h in range(2):
                            hh = 2 * j + h
                            stc = sm.tile([D + 1, NCH], F32, name=f"stc{h}", tag=f"stc{h}")
                            nc.vector.tensor_copy(out=stc, in_=ya[h][:, NCH:TOK])
                            nc.sync.dma_start(out=stash_d[hh, :, :], in_=stc)
                            div_write(ya, h, j, 0, ya[h][0:D, 0:NCH], ya[h][D:D + 1, 0:NCH])

                proj([0, 1])

                # ---- phase B: keys 1024:2047, chunk 1 only ----
                ktb, vb = ktbB, vbB
                ind = build_ind(1, 1)
                with tc.tile_pool(name="at_ps1", bufs=1, space="PSUM") as at_ps:
                    for j in range(8):
                        ya = [at_ps.tile([D + 1, NCH], F32, name=f"ya{h}", tag=f"ya{h}", bufs=1)
                              for h in range(2)]
                        for kt in range(8):
                            st = at_ps.tile([P, 2, TOK], F32, name="st", tag="st", bufs=2)
                            for h in range(2):
                                nc.tensor.matmul(
                                    st[:, h, 0:NCH],
                                    ktb[64 * h:64 * (h + 1), j, kt // 2, (kt % 2) * P:(kt % 2) * P + P],
                                    qT[64 * h:64 * (h + 1), j, NCH:TOK],
                                    start=True, stop=True, tile_position=(64 * h, 0))
                            et = sm.tile([P, 2, NCH], BF16, name="etb", tag="etb", bufs=3)
                            nc.scalar.activation(out=et, in_=st[:, :, 0:NCH], func=AF.Exp, scale=1.0)
                            nc.vector.tensor_tensor(out=et[:, :, :], in0=et[:, :, :],
                                                    in1=ind[:, kt, :, :], op=ALU.mult)
                            for h in range(2):
                                nc.tensor.matmul(ya[h][:, :], vb[:, kt, 2 * j + h, :], et[:, h, :],
                                                 start=(kt == 0), stop=(kt == 7))
                        for h in range(2):
                            hh = 2 * j + h
                            stl = sm.tile([D + 1, NCH], F32, name=f"stl{h}", tag=f"stl{h}", bufs=1)
                            nc.sync.dma_start(out=stl, in_=stash_d[hh, :, :])
                            ysum = sm.tile([D + 1, NCH], F32, name=f"ys{h}", tag=f"ys{h}")
                            nc.vector.tensor_tensor(out=ysum, in0=ya[h][:, :], in1=stl, op=ALU.add)
                            div_write(ya, h, j, 1, ysum[0:D, :], ysum[D:D + 1, :])

                proj([2, 3])

        # ================= LN2 + MLP =================
        with tc.tile_pool(name="mlpp", bufs=1) as mp, \
             tc.tile_pool(name="wmlp", bufs=2) as wmp:
            ln2 = layer_norm(x2, mp, "ln2")
            x28 = transpose_split_to(ln2, mp, "x28")
            for t in range(4):
                nc.vector.tensor_tensor(out=x2[:, t, :], in0=x2[:, t, :], in1=b2_bc, op=ALU.add)

            h_sb = mp.tile([P, 32, 512], BF16)
            w2full = mp.tile([P, 32, C], BF16)
            with tc.tile_pool(name="fc1_ps", bufs=4, space="PSUM") as fc1_ps:
                for g in range(8):
                    wfc = wmp.tile([P, 8, 2, 512], FP8, name="wfc", tag="wfc")
                    nc.sync.dma_start(out=wfc, in_=w_fc8[:, :, :, 512 * g:512 * (g + 1)])
                    for mi in range(4):
                        m = 4 * g + mi
                        fps = fc1_ps.tile([P, 512], F32, name="fps", tag="fps")
                        s2_gemm(fps, wfc, x28, P * mi, P)
                        nc.scalar.activation(out=h_sb[:, m, :], in_=fps[:, :], func=AF.Gelu,
                                             bias=bfc_sb[:, m:m + 1], scale=1.0 / WS)
                    nc.gpsimd.dma_start(out=w2full[:, 4 * g:4 * (g + 1), :],
                                        in_=w_fc2[P * 4 * g:P * 4 * (g + 1), :]
                                        .rearrange("(mc mp) n -> mp mc n", mp=P))
            # fc2 token-tile-outer: residual add + store pipeline out per tile
            with tc.tile_pool(name="fc2_ps", bufs=4, space="PSUM") as fc2_ps:
                for t in range(4):
                    for n in range(2):
                        ops = fc2_ps.tile([P, 512], F32, name="ops", tag="ops")
                        for m in range(32):
                            nc.tensor.matmul(ops[:, :], h_sb[:, m, P * t:P * (t + 1)],
                                             w2full[:, m, 512 * n:512 * (n + 1)],
                                             start=(m == 0), stop=(m == 31))
                        nc.vector.tensor_tensor(out=x2[:, t, 512 * n:512 * (n + 1)],
                                                in0=ops[:, :],
                                                in1=x2[:, t, 512 * n:512 * (n + 1)], op=ALU.add)
                    nc.sync.dma_start(out=out_ext[P * t:P * (t + 1), :], in_=x2[:, t, :])

    nc.finalize()
    return nc


def _get_nc():
    if "nc" not in _CACHE:
        _CACHE["nc"] = _build()
    return _CACHE["nc"]


def _prep(**inputs):
    f = lambda a: np.asarray(a, dtype=np.float32)
    x = f(inputs["x"])
    ln1_g, ln1_b = f(inputs["ln1_g"]), f(inputs["ln1_b"])
    ln2_g, ln2_b = f(inputs["ln2_g"]), f(inputs["ln2_b"])
    W_attn, b_attn = f(inputs["W_attn"]), f(inputs["b_attn"])
    W_o, b_o = f(inputs["W_o"]), f(inputs["b_o"])
    W_fc, b_fc = f(inputs["W_fc"]), f(inputs["b_fc"])
    W_fc2, b_fc2 = f(inputs["W_fc2"]), f(inputs["b_fc2"])

    # fold LN affine params into the next matmul
    W_attn_e = ln1_g[:, None] * W_attn
    b_attn_e = b_attn + ln1_b @ W_attn
    W_fc_e = ln2_g[:, None] * W_fc
    b_fc_e = b_fc + ln2_b @ W_fc

    # fold V bias into b_o (sum of attention weights is 1); pre-scale Q bias
    b_v = b_attn_e[2 * C:3 * C]
    b_o_e = b_o + b_v @ W_o
    b_attn_pass = b_attn_e.copy()
    b_attn_pass[0:C] *= SCALE

    w_attn8 = _pack_w8(W_attn_e)
    w_fc8 = _pack_w8(W_fc_e)

    in_maps = []
    for r in range(N_CORES):
        b, p = divmod(r, 4)
        c0, c1 = p, 7 - p
        xs = np.concatenate([x[b, NCH * c0:NCH * (c0 + 1)],
                             x[b, NCH * c1:NCH * (c1 + 1)]], axis=0)
        in_maps.append({
            "x": np.ascontiguousarray(xs),
            "qbase": np.array([[NCH * c0, NCH * c1]], dtype=np.float32),
            "w_attn8": w_attn8, "b_attn": b_attn_pass,
            "w_o": W_o.astype(ml_dtypes.bfloat16), "b_o": b_o_e,
            "w_fc8": w_fc8, "b_fc": b_fc_e,
            "w_fc2": W_fc2.astype(ml_dtypes.bfloat16), "b_fc2": b_fc2,
        })

    def assemble(results):
        out = np.empty((B, T, C), dtype=np.float32)
        for r in range(N_CORES):
            b, p = divmod(r, 4)
            c0, c1 = p, 7 - p
            o = results[r]["out"]
            out[b, NCH * c0:NCH * (c0 + 1)] = o[0:NCH]
            out[b, NCH * c1:NCH * (c1 + 1)] = o[NCH:TOK]
        return out

    return in_maps, assemble


def kernel(**inputs):
    from concourse.bass_utils import run_bass_kernel_spmd

    in_maps, assemble = _prep(**inputs)
    res = run_bass_kernel_spmd(_get_nc(), in_maps, list(range(N_CORES)))
    return assemble(res.results)
